# revision 5
# baseline (speedup 1.0000x reference)
"""Trainium2 Bass kernel for MultiHeadAttention with Q-dependent RPE bias.

Problem (per full input):
    Q [8, 2048, 256], K [8, 512, 256], V [8, 512, 256], rpe_bias [8, 512, 32]
    dots = einsum(bhid,bhjd->bhij, Qh, Kh)*s + einsum(bhid,hjd->bhij, Qh, rpe)*s
         = s * Qh @ (Kh + rpe)^T          (rpe broadcast over batch)
    out  = softmax(dots, -1) @ Vh
    returns (out [8, 2048, 256], dots [8, 8, 2048, 512])

Sharding: data-parallel over B — each of the 8 NeuronCores handles one batch
element end-to-end (all 8 heads); rpe_bias is replicated. No collectives.

Per-core design:
  - prep: build K'^T = ((K + rpe) * s)^T  [256, 512] and Q^T [256, 2048] in
    SBUF via PE transposes; rows are head-major so head h lives at partition
    strip 32*(h%4) of tile h//4 — exactly the layout needed for 4-way
    row-packed K=32 matmuls. V' = [V_h | ones] per j-chunk ([128, 8, 33]).
  - pass A: dots[i,j] per (head, i-tile): lhsT = Q^T strip [32,128],
    rhs = K'^T strip [32,512] -> PSUM [128,512]; DVE copies PSUM->SBUF
    (DMA cannot read PSUM) and DMA writes the raw scores to DRAM.
  - pass B: dots^T[j,i] recomputed on the PE (row-packed K=32 again) — far
    cheaper than transposing the 8.4M-element attention matrix; ACT applies
    the single exp pass PSUM->SBUF (softmax without max-subtraction is exact
    here: |dots| <~ 10 so exp cannot overflow in fp32).
  - AV: out'[i, h, 0:33] += expT_chunk.T @ V'_chunk over 4 j-chunks; the ones
    column of V' makes row 32 the softmax denominator Z_i. ACT then writes
    out = out' * (1/Z) during the PSUM->SBUF copy (per-partition scale AP),
    with 1/Z from DVE reciprocal.
"""

import numpy as np

B, I, J, D, H = 8, 2048, 512, 256, 8
d = D // H  # 32
SCALE = float(d) ** -0.5
N_CORES = 8
IT = I // 128        # 16 i-tiles of 128 rows
JC = J // 128        # 4 j-chunks of 128
ICH = I // 512       # 4 i-chunks of 512 (pass B moving width)

_cache = {}


def _split_multi_waits(bir_json: bytes) -> bytes:
    """walrus in this image allows only ONE sync-wait per instruction.
    Tile's add_semaphores freely attaches several.  Rewrite the BIR JSON:
    extra waits move onto NoOp instructions inserted just before the
    instruction on the same engine (engine program order preserves the
    gating semantics)."""
    import json

    m = json.loads(bir_json)
    ctr = 0
    for fn in m["functions"]:
        for bb in fn["blocks"]:
            out = []
            for ins in bb["instructions"]:
                si = ins.get("sync_info")
                ow = (si or {}).get("on_wait") or []
                if len(ow) > 1:
                    for w in ow[:-1]:
                        ctr += 1
                        out.append({
                            "engine": ins["engine"],
                            "ins": [], "outs": [],
                            "name": f"I-wsplit-{ctr}",
                            "opcode": "NoOp",
                            "sync_info": {"on_update": [], "on_wait": [w]},
                            "text_hint": "wsplit",
                        })
                    si["on_wait"] = [ow[-1]]
                out.append(ins)
            bb["instructions"] = out
    return json.dumps(m).encode()


def _patch_compile_wait_split():
    import concourse.bass_utils as bu
    import concourse.bass2jax as b2j

    if getattr(bu, "_wsplit_patched", False):
        return
    orig = bu.compile_bir_kernel

    def compile_with_split(bir_json, tmpdir, neff_name="file.neff"):
        return orig(_split_multi_waits(bir_json), tmpdir, neff_name)

    bu.compile_bir_kernel = compile_with_split
    b2j.compile_bir_kernel = compile_with_split
    bu._wsplit_patched = True


def _patch_tile_tail_drain():
    """walrus in this image rejects the Tile tail drain when it carries the
    whole global clock as sem waits ("Too many sync wait commands").  Split
    the waits across one NOP per logical proc before the drain."""
    import concourse.tile as tile
    from concourse.vector_clock import ScopedClock, VectorClock

    def _drain_and_barrier_split(self, tick_clock, wait_clock):
        nc = self.nc
        gc = tick_clock.global_clock
        n = len(gc)
        for proc in range(n):
            t = gc[proc]
            if t == 0:
                continue
            vec = [0] * n
            vec[proc] = t
            nop = nc.sync.nop(nofuse=True, hint=f"tail_wait_p{proc}")
            wait_clock.add_sem_waits(nop.ins, ScopedClock({None: VectorClock(vec)}))
        nc.sync.drain()
        nc.all_engine_barrier()
        assert self.sems is not None
        popped = nc._tile_sem_poison_stack.pop()
        assert popped is self._sem_poison
        nc.clear_and_free_semaphores(list(self.sems.allocated().values()))
        nc.all_engine_barrier()

    tile.TileContext._drain_and_barrier = _drain_and_barrier_split


def build_nc():
    import concourse.bass as bass
    import concourse.tile as tile
    import concourse.mybir as mybir
    from concourse.masks import make_identity

    _patch_tile_tail_drain()
    _patch_compile_wait_split()
    f32 = mybir.dt.float32

    nc = bass.Bass("TRN2", target_bir_lowering=False, debug=False,
                   num_devices=N_CORES)
    q_d = nc.dram_tensor("Q", [I, D], f32, kind="ExternalInput").ap()
    k_d = nc.dram_tensor("K", [J, D], f32, kind="ExternalInput").ap()
    v_d = nc.dram_tensor("V", [J, D], f32, kind="ExternalInput").ap()
    rpe_d = nc.dram_tensor("rpe", [H, J, d], f32, kind="ExternalInput").ap()
    out_d = nc.dram_tensor("out", [I, D], f32, kind="ExternalOutput").ap()
    dots_d = nc.dram_tensor("dots", [H, I, J], f32, kind="ExternalOutput").ap()

    with tile.TileContext(nc) as tc:
        with (
            tc.tile_pool(name="persist", bufs=1) as persist,   # long-lived SBUF
            tc.tile_pool(name="prep", bufs=6) as prep,         # prep staging
            tc.tile_pool(name="expt", bufs=32) as expt_pool,   # exp(dots^T)
            tc.tile_pool(name="dsb", bufs=12) as dsb_pool,     # raw dots staging
            tc.tile_pool(name="osb", bufs=2) as osb_pool,      # out staging
            tc.tile_pool(name="rsb", bufs=2) as rsb_pool,      # 1/Z staging
            tc.tile_pool(name="pa", bufs=4, space="PSUM") as ppa,   # pass A
            tc.tile_pool(name="pb", bufs=3, space="PSUM") as ppb,   # pass B
            tc.tile_pool(name="pav", bufs=1, space="PSUM") as pav,  # AV accum
        ):
            ident = persist.tile([128, 128], f32)
            make_identity(nc, ident[:])

            # ---- prep: K'^T (scaled) and V' --------------------------------
            # kpt[g] [128, 512]: rows = dim 128*g..128*g+127 (heads 4g..4g+3),
            # cols = j.  head h -> tile h//4, partition strip 32*(h%4).
            kpt = [persist.tile([128, J], f32, name=f"kpt{g}") for g in range(2)]
            vp = [persist.tile([128, H, d + 1], f32, name=f"vp{jc}")
                  for jc in range(JC)]
            for jc in range(JC):
                js = slice(jc * 128, (jc + 1) * 128)
                k_sb = prep.tile([128, D], f32, tag="prep_k")
                nc.sync.dma_start(k_sb[:], k_d[js, :])
                r_sb = prep.tile([128, H, d], f32, tag="prep_r")
                nc.sync.dma_start(r_sb[:], rpe_d[:, js, :].rearrange("h p x -> p h x"))
                kp_sb = prep.tile([128, D], f32, tag="prep_kp")
                nc.vector.tensor_add(kp_sb[:], k_sb[:],
                                     r_sb.rearrange("p h x -> p (h x)"))
                # V' chunk: [128, 8, 33]; col 32 of each head = 1.0
                nc.sync.dma_start(vp[jc][:, :, 0:d],
                                  v_d[js, :].rearrange("p (h x) -> p h x", h=H))
                nc.vector.memset(vp[jc][:, :, d:d + 1], 1.0)
                for g in range(2):
                    pt = ppa.tile([128, 512], f32, tag="pa")
                    nc.tensor.transpose(pt[:, 0:128],
                                        kp_sb[:, g * 128:(g + 1) * 128], ident[:])
                    # fold the softmax scale into K'^T here
                    nc.scalar.mul(kpt[g][:, js], pt[:, 0:128], SCALE)

            # ---- prep: Q^T --------------------------------------------------
            qt = [persist.tile([128, I], f32, name=f"qt{g}") for g in range(2)]
            for ti in range(IT):
                isl = slice(ti * 128, (ti + 1) * 128)
                q_sb = prep.tile([128, D], f32, tag="prep_q")
                nc.sync.dma_start(q_sb[:], q_d[isl, :])
                for g in range(2):
                    pt = ppa.tile([128, 512], f32, tag="pa")
                    nc.tensor.transpose(pt[:, 0:128],
                                        q_sb[:, g * 128:(g + 1) * 128], ident[:])
                    nc.vector.tensor_copy(qt[g][:, isl], pt[:, 0:128])

            # ---- main loop over i-chunks of 512 ----------------------------
            for ic in range(ICH):
                ich = slice(ic * 512, (ic + 1) * 512)
                # pass B: dots^T tiles + exp  (row-packed 3/3/2)
                ex = {}
                for jt in range(JC):
                    jsl = slice(jt * 128, (jt + 1) * 128)
                    for group in ((0, 1, 2), (3, 4, 5), (6, 7)):
                        pts = []
                        for h in group:
                            g, s = h // 4, 32 * (h % 4)
                            ssl = slice(s, s + 32)
                            pt = ppb.tile([128, 512], f32, tag="pb")
                            nc.tensor.matmul(pt[:], lhsT=kpt[g][ssl, jsl],
                                             rhs=qt[g][ssl, ich],
                                             start=True, stop=True,
                                             tile_position=(s, 0))
                            pts.append(pt)
                        for h, pt in zip(group, pts):
                            et = expt_pool.tile([128, 512], f32, tag="expt")
                            nc.scalar.activation(
                                et[:], pt[:],
                                mybir.ActivationFunctionType.Exp)
                            ex[(h, jt)] = et

                for it in range(4):
                    ti = ic * 4 + it
                    isl = slice(ti * 128, (ti + 1) * 128)
                    # pass A: raw dots -> SBUF -> DRAM
                    for g in range(2):
                        pts = []
                        for s4 in range(4):
                            h = 4 * g + s4
                            ssl = slice(32 * s4, 32 * s4 + 32)
                            pt = ppa.tile([128, 512], f32, tag="pa")
                            nc.tensor.matmul(pt[:], lhsT=qt[g][ssl, isl],
                                             rhs=kpt[g][ssl, :],
                                             start=True, stop=True,
                                             tile_position=(32 * s4, 0))
                            pts.append(pt)
                        for s4, pt in enumerate(pts):
                            h = 4 * g + s4
                            dt_sb = dsb_pool.tile([128, 512], f32, tag="dsb")
                            nc.vector.tensor_copy(dt_sb[:], pt[:])
                            nc.sync.dma_start(dots_d[h, isl, :], dt_sb[:])
                    # AV: out'[128, h, 33] accumulated over j-chunks
                    av = pav.tile([128, H, d + 1], f32, tag="pav")
                    for h in range(H):
                        for jt in range(JC):
                            nc.tensor.matmul(
                                av[:, h, :],
                                lhsT=ex[(h, jt)][:, it * 128:(it + 1) * 128],
                                rhs=vp[jt][:, h, :],
                                start=(jt == 0), stop=(jt == JC - 1))
                    # normalize by Z (col 32) during PSUM->SBUF copy
                    rz = rsb_pool.tile([128, H], f32, tag="rsb")
                    nc.vector.reciprocal(rz[:], av[:, :, d])
                    o_sb = osb_pool.tile([128, D], f32, tag="osb")
                    for h in range(H):
                        nc.scalar.activation(
                            o_sb[:, h * d:(h + 1) * d], av[:, h, 0:d],
                            mybir.ActivationFunctionType.Copy,
                            scale=rz[:, h:h + 1])
                    nc.sync.dma_start(out_d[isl, :], o_sb[:])
    return nc


def _get_nc():
    if "nc" not in _cache:
        _cache["nc"] = build_nc()
    return _cache["nc"]


def run_spmd(in_maps, **kwargs):
    from concourse.bass_utils import run_bass_kernel_spmd
    return run_bass_kernel_spmd(_get_nc(), in_maps,
                                core_ids=list(range(N_CORES)), **kwargs)


def kernel(Q, K, V, rpe_bias):
    Q = np.ascontiguousarray(np.asarray(Q, dtype=np.float32))
    K = np.ascontiguousarray(np.asarray(K, dtype=np.float32))
    V = np.ascontiguousarray(np.asarray(V, dtype=np.float32))
    rpe = np.ascontiguousarray(np.asarray(rpe_bias, dtype=np.float32))
    in_maps = [
        {"Q": Q[b], "K": K[b], "V": V[b], "rpe": rpe}
        for b in range(B)
    ]
    res = run_spmd(in_maps)
    out = np.stack([res.results[b]["out"] for b in range(B)])
    dots = np.stack([res.results[b]["dots"] for b in range(B)])
    return out, dots


# revision 6
# speedup vs baseline: 2.1380x; 2.1380x over previous
"""Trainium2 Bass kernel for MultiHeadAttention with Q-dependent RPE bias.

Problem (per full input):
    Q [8, 2048, 256], K [8, 512, 256], V [8, 512, 256], rpe_bias [8, 512, 32]
    dots = einsum(bhid,bhjd->bhij, Qh, Kh)*s + einsum(bhid,hjd->bhij, Qh, rpe)*s
         = s * Qh @ (Kh + rpe)^T          (rpe broadcast over batch)
    out  = softmax(dots, -1) @ Vh
    returns (out [8, 2048, 256], dots [8, 8, 2048, 512])

Sharding: data-parallel over B — each of the 8 NeuronCores handles one batch
element end-to-end (all 8 heads); rpe_bias is replicated. No collectives.

Per-core design (fp32 matmuls on trn2 run in LOW_HIGH 2-pass mode, ~4 cyc per
moving column, so the structure minimizes matmul instructions and streams):
  - prep: K'^T = ((K + rpe) * s)^T [256, 512] and Q^T [256, 2048] in SBUF via
    PE transposes; rows head-major so head h sits at partition strip 32*(h%4)
    of tile h//4 — the layout needed for 4-way row-packed K=32 matmuls.
    V' = [V_h | ones] per j-chunk ([128, 8, 33]).
  - scores: dots^T[j, i] per (head, j-tile, i-chunk): lhsT = K'^T strip
    [32, 128], rhs = Q^T strip [32, 512] -> PSUM [128, 512], 4 heads
    row-packed concurrently.  DVE/ACT copy PSUM->SBUF (DMA cannot read PSUM)
    and DMA writes the raw scores as dots_t [H, J, I]; the host transposes to
    [H, I, J] during unshard.  Computing the scores only in [j, i] layout
    (softmax layout) halves the PE work vs also materializing [i, j].
  - softmax: ACT applies the single exp pass PSUM->SBUF (no max-subtraction:
    |dots| <~ 10 so fp32 exp cannot overflow).
  - AV: out'[dd, i] += V'_chunk.T @ expT_chunk over 4 j-chunks: lhsT = V'_h
    [128, 33] (stationary, reused), rhs = expT [128, 512] (moving).  Two heads
    col-packed per PSUM bank via tile_position (0,0)/(0,64).  The ones column
    of V' makes row 32 the softmax denominator Z_i.  Written as out_t
    [H, 33, I]; the host divides by Z and transposes during unshard (0.4% of
    the FLOPs; all matmuls/exp stay on device).
"""

import numpy as np

B, I, J, D, H = 8, 2048, 512, 256, 8
d = D // H  # 32
SCALE = float(d) ** -0.5
N_CORES = 8
IT = I // 128        # 16 i-tiles of 128 rows
JC = J // 128        # 4 j-chunks of 128
ICH = I // 512       # 4 i-chunks of 512 (moving width)

_cache = {}


def _split_multi_waits(bir_json: bytes) -> bytes:
    """walrus in this image allows only ONE sync-wait per instruction.
    Tile's add_semaphores freely attaches several.  Rewrite the BIR JSON:
    extra waits move onto NoOp instructions inserted just before the
    instruction on the same engine (engine program order preserves the
    gating semantics)."""
    import json

    m = json.loads(bir_json)
    ctr = 0
    for fn in m["functions"]:
        for bb in fn["blocks"]:
            out = []
            for ins in bb["instructions"]:
                si = ins.get("sync_info")
                ow = (si or {}).get("on_wait") or []
                if len(ow) > 1:
                    for w in ow[:-1]:
                        ctr += 1
                        out.append({
                            "engine": ins["engine"],
                            "ins": [], "outs": [],
                            "name": f"I-wsplit-{ctr}",
                            "opcode": "NoOp",
                            "sync_info": {"on_update": [], "on_wait": [w]},
                            "text_hint": "wsplit",
                        })
                    si["on_wait"] = [ow[-1]]
                out.append(ins)
            bb["instructions"] = out
    return json.dumps(m).encode()


def _patch_compile_wait_split():
    import concourse.bass_utils as bu
    import concourse.bass2jax as b2j

    if getattr(bu, "_wsplit_patched", False):
        return
    orig = bu.compile_bir_kernel

    def compile_with_split(bir_json, tmpdir, neff_name="file.neff"):
        return orig(_split_multi_waits(bir_json), tmpdir, neff_name)

    bu.compile_bir_kernel = compile_with_split
    b2j.compile_bir_kernel = compile_with_split
    bu._wsplit_patched = True


def _patch_tile_tail_drain():
    """Split the Tile tail drain's global-clock waits across one NOP per
    proc (same walrus single-wait limit as above)."""
    import concourse.tile as tile
    from concourse.vector_clock import ScopedClock, VectorClock

    def _drain_and_barrier_split(self, tick_clock, wait_clock):
        nc = self.nc
        gc = tick_clock.global_clock
        n = len(gc)
        for proc in range(n):
            t = gc[proc]
            if t == 0:
                continue
            vec = [0] * n
            vec[proc] = t
            nop = nc.sync.nop(nofuse=True, hint=f"tail_wait_p{proc}")
            wait_clock.add_sem_waits(nop.ins, ScopedClock({None: VectorClock(vec)}))
        nc.sync.drain()
        nc.all_engine_barrier()
        assert self.sems is not None
        popped = nc._tile_sem_poison_stack.pop()
        assert popped is self._sem_poison
        nc.clear_and_free_semaphores(list(self.sems.allocated().values()))
        nc.all_engine_barrier()

    tile.TileContext._drain_and_barrier = _drain_and_barrier_split


def build_nc():
    import concourse.bass as bass
    import concourse.tile as tile
    import concourse.mybir as mybir
    from concourse.masks import make_identity

    _patch_tile_tail_drain()
    _patch_compile_wait_split()
    f32 = mybir.dt.float32

    nc = bass.Bass("TRN2", target_bir_lowering=False, debug=False,
                   num_devices=N_CORES)
    q_d = nc.dram_tensor("Q", [I, D], f32, kind="ExternalInput").ap()
    k_d = nc.dram_tensor("K", [J, D], f32, kind="ExternalInput").ap()
    v_d = nc.dram_tensor("V", [J, D], f32, kind="ExternalInput").ap()
    rpe_d = nc.dram_tensor("rpe", [H, J, d], f32, kind="ExternalInput").ap()
    outt_d = nc.dram_tensor("out_t", [H, d + 1, I], f32,
                            kind="ExternalOutput").ap()
    dott_d = nc.dram_tensor("dots_t", [H, J, I], f32,
                            kind="ExternalOutput").ap()

    with tile.TileContext(nc) as tc:
        with (
            tc.tile_pool(name="persist", bufs=1) as persist,   # long-lived SBUF
            tc.tile_pool(name="prep", bufs=3) as prep,         # prep staging
            tc.tile_pool(name="expt", bufs=32) as expt_pool,   # exp(dots^T)
            tc.tile_pool(name="dsb", bufs=12) as dsb_pool,     # raw dots staging
            tc.tile_pool(name="osb", bufs=4) as osb_pool,      # out' staging
            tc.tile_pool(name="pb", bufs=4, space="PSUM") as ppb,   # scores
            tc.tile_pool(name="pav", bufs=4, space="PSUM") as pav,  # AV accum
        ):
            ident = persist.tile([128, 128], f32)
            make_identity(nc, ident[:])

            # ---- prep: K'^T (scaled) and V' --------------------------------
            # kpt[g] [128, 512]: rows = dim 128*g..128*g+127 (heads 4g..4g+3),
            # cols = j.  head h -> tile h//4, partition strip 32*(h%4).
            kpt = [persist.tile([128, J], f32, name=f"kpt{g}") for g in range(2)]
            vp = [persist.tile([128, H, d + 1], f32, name=f"vp{jc}")
                  for jc in range(JC)]
            for jc in range(JC):
                js = slice(jc * 128, (jc + 1) * 128)
                k_sb = prep.tile([128, D], f32, tag="prep_k")
                nc.sync.dma_start(k_sb[:], k_d[js, :])
                r_sb = prep.tile([128, H, d], f32, tag="prep_r")
                nc.sync.dma_start(r_sb[:], rpe_d[:, js, :].rearrange("h p x -> p h x"))
                kp_sb = prep.tile([128, D], f32, tag="prep_kp")
                nc.vector.tensor_add(kp_sb[:], k_sb[:],
                                     r_sb.rearrange("p h x -> p (h x)"))
                nc.sync.dma_start(vp[jc][:, :, 0:d],
                                  v_d[js, :].rearrange("p (h x) -> p h x", h=H))
                nc.vector.memset(vp[jc][:, :, d:d + 1], 1.0)
                for g in range(2):
                    pt = ppb.tile([128, 512], f32, tag="pb")
                    nc.tensor.transpose(pt[:, 0:128],
                                        kp_sb[:, g * 128:(g + 1) * 128], ident[:])
                    # fold the softmax scale into K'^T here
                    nc.scalar.mul(kpt[g][:, js], pt[:, 0:128], SCALE)

            # ---- prep: Q^T --------------------------------------------------
            qt = [persist.tile([128, I], f32, name=f"qt{g}") for g in range(2)]
            for ti in range(IT):
                isl = slice(ti * 128, (ti + 1) * 128)
                q_sb = prep.tile([128, D], f32, tag="prep_q")
                nc.sync.dma_start(q_sb[:], q_d[isl, :])
                for g in range(2):
                    pt = ppb.tile([128, 512], f32, tag="pb")
                    nc.tensor.transpose(pt[:, 0:128],
                                        q_sb[:, g * 128:(g + 1) * 128], ident[:])
                    nc.vector.tensor_copy(qt[g][:, isl], pt[:, 0:128])

            # ---- main loop over i-chunks of 512 ----------------------------
            for ic in range(ICH):
                ich = slice(ic * 512, (ic + 1) * 512)
                # scores: dots^T tiles (4 heads row-packed) + exp + raw DMA out
                ex = {}
                for jt in range(JC):
                    jsl = slice(jt * 128, (jt + 1) * 128)
                    for g in range(2):
                        pts = []
                        for s4 in range(4):
                            h = 4 * g + s4
                            ssl = slice(32 * s4, 32 * s4 + 32)
                            pt = ppb.tile([128, 512], f32, tag="pb")
                            nc.tensor.matmul(pt[:], lhsT=kpt[g][ssl, jsl],
                                             rhs=qt[g][ssl, ich],
                                             start=True, stop=True,
                                             tile_position=(32 * s4, 0))
                            pts.append(pt)
                        for s4, pt in enumerate(pts):
                            h = 4 * g + s4
                            # raw scores -> SBUF -> DRAM (split DVE/ACT by head
                            # to balance engines: ACT also does exp+out copies)
                            dt_sb = dsb_pool.tile([128, 512], f32, tag="dsb")
                            if s4 % 4 == 3:
                                nc.scalar.copy(dt_sb[:], pt[:])
                            else:
                                nc.vector.tensor_copy(dt_sb[:], pt[:])
                            nc.sync.dma_start(dott_d[h, jsl, ich], dt_sb[:])
                            et = expt_pool.tile([128, 512], f32, tag="expt")
                            nc.scalar.activation(
                                et[:], pt[:],
                                mybir.ActivationFunctionType.Exp)
                            ex[(h, jt)] = et

                # AV: two heads col-packed per PSUM bank
                for pr in range(4):
                    h0, h1 = 2 * pr, 2 * pr + 1
                    av = pav.tile([128, 512], f32, tag="pav")
                    for jt in range(JC):
                        st, sp = (jt == 0), (jt == JC - 1)
                        nc.tensor.matmul(av[0:d + 1, :],
                                         lhsT=vp[jt][:, h0, :],
                                         rhs=ex[(h0, jt)][:],
                                         start=st, stop=sp,
                                         tile_position=(0, 0))
                        nc.tensor.matmul(av[64:64 + d + 1, :],
                                         lhsT=vp[jt][:, h1, :],
                                         rhs=ex[(h1, jt)][:],
                                         start=st, stop=sp,
                                         tile_position=(0, 64))
                    for k, h in ((0, h0), (64, h1)):
                        o_sb = osb_pool.tile([d + 1, 512], f32, tag="osb")
                        nc.scalar.copy(o_sb[:], av[k:k + d + 1, :])
                        nc.sync.dma_start(outt_d[h, :, ich], o_sb[:])
    return nc


def _get_nc():
    if "nc" not in _cache:
        _cache["nc"] = build_nc()
    return _cache["nc"]


def run_spmd(in_maps, **kwargs):
    from concourse.bass_utils import run_bass_kernel_spmd
    return run_bass_kernel_spmd(_get_nc(), in_maps,
                                core_ids=list(range(N_CORES)), **kwargs)


def assemble(results):
    """Host-side unshard: transpose dots_t -> dots and normalize out_t."""
    out = np.empty((B, I, D), dtype=np.float32)
    dots = np.empty((B, H, I, J), dtype=np.float32)
    for b in range(B):
        ot = results[b]["out_t"]                    # [H, 33, I]
        p = ot[:, :d, :] / ot[:, d:d + 1, :]        # [H, 32, I]
        out[b] = p.transpose(2, 0, 1).reshape(I, D)
        dots[b] = results[b]["dots_t"].transpose(0, 2, 1)
    return out, dots


def kernel(Q, K, V, rpe_bias):
    Q = np.ascontiguousarray(np.asarray(Q, dtype=np.float32))
    K = np.ascontiguousarray(np.asarray(K, dtype=np.float32))
    V = np.ascontiguousarray(np.asarray(V, dtype=np.float32))
    rpe = np.ascontiguousarray(np.asarray(rpe_bias, dtype=np.float32))
    in_maps = [
        {"Q": Q[b], "K": K[b], "V": V[b], "rpe": rpe}
        for b in range(B)
    ]
    res = run_spmd(in_maps)
    return assemble(res.results)


# revision 10
# speedup vs baseline: 2.9518x; 1.3806x over previous
"""Trainium2 Bass kernel for MultiHeadAttention with Q-dependent RPE bias.

Problem (per full input):
    Q [8, 2048, 256], K [8, 512, 256], V [8, 512, 256], rpe_bias [8, 512, 32]
    dots = einsum(bhid,bhjd->bhij, Qh, Kh)*s + einsum(bhid,hjd->bhij, Qh, rpe)*s
         = s * Qh @ (Kh + rpe)^T          (rpe broadcast over batch)
    out  = softmax(dots, -1) @ Vh
    returns (out [8, 2048, 256], dots [8, 8, 2048, 512])

Sharding: data-parallel over B — each of the 8 NeuronCores handles one batch
element end-to-end (all 8 heads); rpe_bias is replicated. No collectives.

Per-core design (fp32 matmuls on trn2 run in LOW_HIGH 2-pass mode, ~4 cyc per
moving column, so the structure minimizes matmul instructions and streams):
  - prep: K'^T = ((K + rpe) * s)^T [256, 512] and Q^T [256, 2048] in SBUF via
    PE transposes; rows head-major so head h sits at partition strip 32*(h%4)
    of tile h//4 — the layout needed for 4-way row-packed K=32 matmuls.
    V' = [V_h | ones] per j-chunk ([128, 8, 33]).
  - scores: dots^T[j, i] per (head, j-tile, i-chunk): lhsT = K'^T strip
    [32, 128], rhs = Q^T strip [32, 512] -> PSUM [128, 512], 4 heads
    row-packed concurrently.  DVE/ACT copy PSUM->SBUF (DMA cannot read PSUM)
    and DMA writes the raw scores as dots_t [H, J, I]; the host transposes to
    [H, I, J] during unshard.  Computing the scores only in [j, i] layout
    (softmax layout) halves the PE work vs also materializing [i, j].
  - softmax: ACT applies the single exp pass PSUM->SBUF (no max-subtraction:
    |dots| <~ 10 so fp32 exp cannot overflow).
  - AV: out'[dd, i] += V'_chunk.T @ expT_chunk over 4 j-chunks: lhsT = V'_h
    [128, 33] (stationary, reused), rhs = expT [128, 512] (moving).  Two heads
    col-packed per PSUM bank via tile_position (0,0)/(0,64).  The ones column
    of V' makes row 32 the softmax denominator Z_i.  Written as out_t
    [H, 33, I]; the host divides by Z and transposes during unshard (0.4% of
    the FLOPs; all matmuls/exp stay on device).
"""

import numpy as np

B, I, J, D, H = 8, 2048, 512, 256, 8
d = D // H  # 32
SCALE = float(d) ** -0.5
N_CORES = 8
IT = I // 128        # 16 i-tiles of 128 rows
JC = J // 128        # 4 j-chunks of 128
ICH = I // 512       # 4 i-chunks of 512 (moving width)

_cache = {}


def _split_multi_waits(bir_json: bytes) -> bytes:
    """walrus in this image allows only ONE sync-wait per instruction.
    Tile's add_semaphores freely attaches several.  Rewrite the BIR JSON:
    extra waits move onto NoOp instructions inserted just before the
    instruction on the same engine (engine program order preserves the
    gating semantics)."""
    import json

    m = json.loads(bir_json)
    ctr = 0
    for fn in m["functions"]:
        for bb in fn["blocks"]:
            out = []
            for ins in bb["instructions"]:
                si = ins.get("sync_info")
                ow = (si or {}).get("on_wait") or []
                if len(ow) > 1:
                    for w in ow[:-1]:
                        ctr += 1
                        out.append({
                            "engine": ins["engine"],
                            "ins": [], "outs": [],
                            "name": f"I-wsplit-{ctr}",
                            "opcode": "NoOp",
                            "sync_info": {"on_update": [], "on_wait": [w]},
                            "text_hint": "wsplit",
                        })
                    si["on_wait"] = [ow[-1]]
                out.append(ins)
            bb["instructions"] = out
    return json.dumps(m).encode()


def _patch_compile_wait_split():
    import concourse.bass_utils as bu
    import concourse.bass2jax as b2j

    if getattr(bu, "_wsplit_patched", False):
        return
    orig = bu.compile_bir_kernel

    def compile_with_split(bir_json, tmpdir, neff_name="file.neff"):
        return orig(_split_multi_waits(bir_json), tmpdir, neff_name)

    bu.compile_bir_kernel = compile_with_split
    b2j.compile_bir_kernel = compile_with_split
    bu._wsplit_patched = True


def _patch_tile_tail_drain():
    """Split the Tile tail drain's global-clock waits across one NOP per
    proc (same walrus single-wait limit as above)."""
    import concourse.tile as tile
    from concourse.vector_clock import ScopedClock, VectorClock

    def _drain_and_barrier_split(self, tick_clock, wait_clock):
        nc = self.nc
        gc = tick_clock.global_clock
        n = len(gc)
        for proc in range(n):
            t = gc[proc]
            if t == 0:
                continue
            vec = [0] * n
            vec[proc] = t
            nop = nc.sync.nop(nofuse=True, hint=f"tail_wait_p{proc}")
            wait_clock.add_sem_waits(nop.ins, ScopedClock({None: VectorClock(vec)}))
        nc.sync.drain()
        nc.all_engine_barrier()
        assert self.sems is not None
        popped = nc._tile_sem_poison_stack.pop()
        assert popped is self._sem_poison
        nc.clear_and_free_semaphores(list(self.sems.allocated().values()))
        nc.all_engine_barrier()

    tile.TileContext._drain_and_barrier = _drain_and_barrier_split


def build_nc():
    import concourse.bass as bass
    import concourse.tile as tile
    import concourse.mybir as mybir
    from concourse.masks import make_identity

    _patch_tile_tail_drain()
    _patch_compile_wait_split()
    f32 = mybir.dt.float32

    nc = bass.Bass("TRN2", target_bir_lowering=False, debug=False,
                   num_devices=N_CORES)
    q_d = nc.dram_tensor("Q", [I, D], f32, kind="ExternalInput").ap()
    k_d = nc.dram_tensor("K", [J, D], f32, kind="ExternalInput").ap()
    v_d = nc.dram_tensor("V", [J, D], f32, kind="ExternalInput").ap()
    rpe_d = nc.dram_tensor("rpe", [H, J, d], f32, kind="ExternalInput").ap()
    outt_d = nc.dram_tensor("out_t", [H, d + 1, I], f32,
                            kind="ExternalOutput").ap()
    dott_d = nc.dram_tensor("dots_t", [H, J, I], f32,
                            kind="ExternalOutput").ap()

    with tile.TileContext(nc) as tc:
        with (
            tc.tile_pool(name="persist", bufs=1) as persist,   # long-lived SBUF
            tc.tile_pool(name="prep", bufs=3) as prep,         # prep staging
            tc.tile_pool(name="expt", bufs=36) as expt_pool,   # exp(dots^T)
            tc.tile_pool(name="dsb", bufs=8) as dsb_pool,      # raw dots staging
            tc.tile_pool(name="osb", bufs=4) as osb_pool,      # out' staging
            tc.tile_pool(name="ps", bufs=8, space="PSUM") as pps,  # all 8 banks
        ):
            ppb = pps
            ident = persist.tile([128, 128], f32)
            make_identity(nc, ident[:])

            # ---- prep: K'^T (scaled) and V' --------------------------------
            # kpt[g] [128, 512]: rows = dim 128*g..128*g+127 (heads 4g..4g+3),
            # cols = j.  head h -> tile h//4, partition strip 32*(h%4).
            kpt = [persist.tile([128, J], f32, name=f"kpt{g}") for g in range(2)]
            vp = [persist.tile([128, H, d + 1], f32, name=f"vp{jc}")
                  for jc in range(JC)]
            for jc in range(JC):
                js = slice(jc * 128, (jc + 1) * 128)
                k_sb = prep.tile([128, D], f32, tag="prep_k")
                nc.sync.dma_start(k_sb[:], k_d[js, :])
                r_sb = prep.tile([128, H, d], f32, tag="prep_r")
                nc.sync.dma_start(r_sb[:], rpe_d[:, js, :].rearrange("h p x -> p h x"))
                kp_sb = prep.tile([128, D], f32, tag="prep_kp")
                nc.vector.tensor_add(kp_sb[:], k_sb[:],
                                     r_sb.rearrange("p h x -> p (h x)"))
                nc.sync.dma_start(vp[jc][:, :, 0:d],
                                  v_d[js, :].rearrange("p (h x) -> p h x", h=H))
                nc.vector.memset(vp[jc][:, :, d:d + 1], 1.0)
                for g in range(2):
                    pt = ppb.tile([128, 512], f32, tag="ps")
                    nc.tensor.transpose(pt[:, 0:128],
                                        kp_sb[:, g * 128:(g + 1) * 128], ident[:])
                    # fold the softmax scale into K'^T here
                    nc.scalar.mul(kpt[g][:, js], pt[:, 0:128], SCALE)

            # ---- prep: Q^T --------------------------------------------------
            qt = [persist.tile([128, I], f32, name=f"qt{g}") for g in range(2)]
            for ti in range(IT):
                isl = slice(ti * 128, (ti + 1) * 128)
                q_sb = prep.tile([128, D], f32, tag="prep_q")
                nc.sync.dma_start(q_sb[:], q_d[isl, :])
                for g in range(2):
                    pt = ppb.tile([128, 512], f32, tag="ps")
                    nc.tensor.transpose(pt[:, 0:128],
                                        q_sb[:, g * 128:(g + 1) * 128], ident[:])
                    nc.vector.tensor_copy(qt[g][:, isl], pt[:, 0:128])

            # ---- main loop: software-pipelined halves (ic, head-group) -----
            # Each half = 4 score bursts (4 row-packed MMs each) for heads
            # 4g..4g+3.  The previous half's AV pairs are emitted between this
            # half's bursts so the PE instruction stream stays dense (keeps
            # the HAM clock-gate warm) and PSUM drains overlap matmuls.
            ex = {}

            def emit_burst(ic, g, jt):
                ich = slice(ic * 512, (ic + 1) * 512)
                jsl = slice(jt * 128, (jt + 1) * 128)
                pts = []
                for s4 in range(4):
                    ssl = slice(32 * s4, 32 * s4 + 32)
                    pt = ppb.tile([128, 512], f32, tag="ps")
                    nc.tensor.matmul(pt[:], lhsT=kpt[g][ssl, jsl],
                                     rhs=qt[g][ssl, ich],
                                     start=True, stop=True,
                                     tile_position=(32 * s4, 0))
                    pts.append(pt)
                for s4, pt in enumerate(pts):
                    h = 4 * g + s4
                    dt_sb = dsb_pool.tile([128, 512], f32, tag="dsb")
                    nc.vector.tensor_copy(dt_sb[:], pt[:])
                    # split DMA issue across the two DGE paths
                    eng = nc.sync if (jt % 2 == 0) else nc.gpsimd
                    eng.dma_start(dott_d[h, jsl, ich], dt_sb[:])
                    et = expt_pool.tile([128, 512], f32, tag="expt")
                    nc.scalar.activation(et[:], pt[:],
                                         mybir.ActivationFunctionType.Exp)
                    ex[(h, jt)] = et

            def emit_av_pair(ic, g, p):
                # pair p in {0,1} of head-group g: heads (4g+2p, 4g+2p+1)
                ich = slice(ic * 512, (ic + 1) * 512)
                h0, h1 = 4 * g + 2 * p, 4 * g + 2 * p + 1
                av = ppb.tile([128, 512], f32, tag="ps")
                for jt in range(JC):
                    st, sp = (jt == 0), (jt == JC - 1)
                    nc.tensor.matmul(av[0:d + 1, :],
                                     lhsT=vp[jt][:, h0, :],
                                     rhs=ex[(h0, jt)][:],
                                     start=st, stop=sp,
                                     tile_position=(0, 0))
                    nc.tensor.matmul(av[64:64 + d + 1, :],
                                     lhsT=vp[jt][:, h1, :],
                                     rhs=ex[(h1, jt)][:],
                                     start=st, stop=sp,
                                     tile_position=(0, 64))
                for k, h in ((0, h0), (64, h1)):
                    o_sb = osb_pool.tile([d + 1, 512], f32, tag="osb")
                    nc.scalar.copy(o_sb[:], av[k:k + d + 1, :])
                    nc.sync.dma_start(outt_d[h, :, ich], o_sb[:])

            halves = [(ic, g) for ic in range(ICH) for g in range(2)]
            prev = None
            for ic, g in halves:
                for jt in range(JC):
                    emit_burst(ic, g, jt)
                    if prev is not None and jt in (1, 3):
                        emit_av_pair(prev[0], prev[1], (jt - 1) // 2)
                prev = (ic, g)
            emit_av_pair(prev[0], prev[1], 0)
            emit_av_pair(prev[0], prev[1], 1)
    return nc


def _get_nc():
    if "nc" not in _cache:
        _cache["nc"] = build_nc()
    return _cache["nc"]


def run_spmd(in_maps, **kwargs):
    from concourse.bass_utils import run_bass_kernel_spmd
    return run_bass_kernel_spmd(_get_nc(), in_maps,
                                core_ids=list(range(N_CORES)), **kwargs)


def assemble(results):
    """Host-side unshard: transpose dots_t -> dots and normalize out_t."""
    out = np.empty((B, I, D), dtype=np.float32)
    dots = np.empty((B, H, I, J), dtype=np.float32)
    for b in range(B):
        ot = results[b]["out_t"]                    # [H, 33, I]
        p = ot[:, :d, :] / ot[:, d:d + 1, :]        # [H, 32, I]
        out[b] = p.transpose(2, 0, 1).reshape(I, D)
        dots[b] = results[b]["dots_t"].transpose(0, 2, 1)
    return out, dots


def kernel(Q, K, V, rpe_bias):
    Q = np.ascontiguousarray(np.asarray(Q, dtype=np.float32))
    K = np.ascontiguousarray(np.asarray(K, dtype=np.float32))
    V = np.ascontiguousarray(np.asarray(V, dtype=np.float32))
    rpe = np.ascontiguousarray(np.asarray(rpe_bias, dtype=np.float32))
    in_maps = [
        {"Q": Q[b], "K": K[b], "V": V[b], "rpe": rpe}
        for b in range(B)
    ]
    res = run_spmd(in_maps)
    return assemble(res.results)
